# revision 28
# baseline (speedup 1.0000x reference)
"""KDE2D Trainium2 Bass kernel — nearest-binned separable formulation.

Reference (per (b,t), B=16, T=64, N=512, grid 128x128, h=bandwidth):
  standardize points (mean/std ddof=1 over N), then
  density[gx,gy] = norm * sum_n exp(-c(xg[gx]-x_n)^2) * exp(-c(yg[gy]-y_n)^2)

Kernel strategy (data-parallel over 1024 (b,t) pairs, 128 per core):
  Nearest-neighbour binning on an auxiliary S=96 grid s[-5.25, 5.25]:
    density ~= K1^T W K2,  W[i,j] = #{n : ix_n==i, iy_n==j},
    K1[i,g] = exp(-c(s_i-g_g)^2), K2 = K1*norm  (constants).
  Measured rel-Frobenius error of the approximation vs the exact
  reference is 6.7e-3 (tolerance 2e-2).

  Per (b,t): one-hot tiles U[cc][n(128 part), S] = (iota==idx_n) built by
  DVE tensor_scalar(is_equal) in 4x bf16 mode (94ns) / GPSIMD for one
  chunk pair; W accumulated over 4 n-chunks by PE; then two constant
  matmuls (st=W, mv=K1) -> U, (st=U, mv=K2) -> density. Four (b,t) share
  each PSUM bank so the PSUM->SBUF copies (ACT) are [*, 4*tile] wide.
  Output DMA batched 8 bt per descriptor-set.
"""

import math

import numpy as np
from ml_dtypes import bfloat16

import concourse.bass as bass
import concourse.bacc as bacc
import concourse.mybir as mybir
from concourse import tile
from concourse.bass_utils import run_bass_kernel_spmd

B, T, N, GRID = 16, 64, 512, 128
NCORES = 8
BT_PER_CORE = (B * T) // NCORES  # 128
NCHUNK = N // 128  # 4
S = 96           # auxiliary binning grid size
HALF = 5.25      # auxiliary grid spans [-HALF, HALF]
DS = 2.0 * HALF / (S - 1)

F32 = mybir.dt.float32
BF16 = mybir.dt.bfloat16

_CACHE = {}


def _build(bw: float):
    nc = bacc.Bacc("TRN2", target_bir_lowering=False)
    a_ext = nc.declare_dram_parameter("a", [BT_PER_CORE, N, 2], F32, isOutput=False)
    iota_ext = nc.declare_dram_parameter("iota", [128, S], BF16, isOutput=False)
    k1_ext = nc.declare_dram_parameter("k1", [S, GRID], BF16, isOutput=False)
    k2_ext = nc.declare_dram_parameter("k2", [S, GRID], BF16, isOutput=False)
    idt_ext = nc.declare_dram_parameter("idt", [128, 128], F32, isOutput=False)
    # out[g2, gx, (half,k,gy)] ; host reshapes to [128bt, 128, 128]
    out_ext = nc.declare_dram_parameter(
        "out", [BT_PER_CORE // 8, GRID, 8 * GRID], F32, isOutput=True
    )

    AT = mybir.ActivationFunctionType
    OP = mybir.AluOpType

    with tile.TileContext(nc) as tc:
        with (
            tc.tile_pool(name="const", bufs=1) as cpool,
            tc.tile_pool(name="stats", bufs=1) as spool,
            tc.tile_pool(name="work", bufs=4) as wpool,
            tc.tile_pool(name="oh", bufs=24) as ohpool,
            tc.tile_pool(name="ohp", bufs=24) as ohppool,
            tc.tile_pool(name="psumW", bufs=2, space="PSUM") as wppool,
            tc.tile_pool(name="psumU", bufs=3, space="PSUM") as uppool,
            tc.tile_pool(name="psumD", bufs=3, space="PSUM") as dppool,
            tc.tile_pool(name="wsb", bufs=4) as wsbpool,
            tc.tile_pool(name="usb", bufs=5) as usbpool,
            tc.tile_pool(name="outp", bufs=4) as opool,
        ):
            # split the input load across two DGE queues (SP + ACT) so the
            # halves transfer in parallel and stats can start on half 1
            a_all = spool.tile([128, N, 2], F32, tag="a")
            NH = N // 2
            nc.sync.dma_start(a_all[:, 0:NH, :], a_ext[:, 0:NH])
            nc.scalar.dma_start(a_all[:, NH:N, :], a_ext[:, NH:N])
            iota_sb = cpool.tile([128, S], BF16, tag="iota")
            k1_sb = cpool.tile([S, GRID], BF16, tag="k1")
            k2_sb = cpool.tile([S, GRID], BF16, tag="k2")
            idt_sb = cpool.tile([128, 128], F32, tag="idt")
            nc.sync.dma_start(iota_sb[:], iota_ext[:])
            nc.sync.dma_start(k1_sb[:], k1_ext[:])
            nc.sync.dma_start(k2_sb[:], k2_ext[:])
            nc.sync.dma_start(idt_sb[:], idt_ext[:])

            # ---- per-bt stats -> bin indices (layout [bt(128 part), n]) ----
            # t = ((x-mean)*invsd + HALF)/DS + 0.5 ; idx = floor(clip(t))
            #   = x*A + Bc with A = invsd/DS, Bc = (HALF - mean*invsd)/DS + .5
            dummy_pt = dppool.tile([GRID, 4 * GRID], F32, tag="dps")
            nc.tensor.transpose(dummy_pt[:, 0:128], idt_sb[:], idt_sb[:])
            probe = spool.tile([128, 1], F32, tag="probe")
            nc.scalar.activation(probe[:], iota_sb[:, 0:1], AT.Copy)
            idxT = {"x": [], "y": []}
            for ch, ci in (("x", 0), ("y", 1)):
                src = a_all[:, :, ci]
                bn = spool.tile([128, 2, 6], F32, tag=f"bn{ch}")
                nc.vector.bn_stats(bn[:, 0, :], a_all[:, 0:NH, ci])
                nc.vector.bn_stats(bn[:, 1, :], a_all[:, NH:N, ci])
                mv = spool.tile([128, 2], F32, tag=f"mv{ch}")
                nc.vector.bn_aggr(mv[:], bn[:])
                # invsd = 1/sqrt(var_pop * N/(N-1))  (ddof=1)
                sd = spool.tile([128, 1], F32, tag=f"sd{ch}")
                nc.scalar.activation(
                    sd[:], mv[:, 1:2], AT.Sqrt, scale=float(N) / (N - 1)
                )
                invsd = spool.tile([128, 1], F32, tag=f"invsd{ch}")
                nc.vector.reciprocal(invsd[:], sd[:])
                av = spool.tile([128, 1], F32, tag=f"av{ch}")
                nc.vector.tensor_scalar_mul(av[:], invsd[:], 1.0 / DS)
                mb = spool.tile([128, 1], F32, tag=f"mb{ch}")
                nc.vector.tensor_tensor(mb[:], mv[:, 0:1], av[:], OP.mult)
                bv = spool.tile([128, 1], F32, tag=f"bv{ch}")
                nc.vector.tensor_scalar(
                    bv[:], mb[:], -1.0, HALF / DS, OP.mult, OP.add
                )
                tv = wpool.tile([128, N], F32, tag=f"tv{ch}")
                nc.vector.tensor_scalar(
                    tv[:], src, av[:, 0:1], bv[:, 0:1], OP.mult, OP.add
                )
                nc.vector.tensor_scalar(
                    tv[:], tv[:], 0.0, float(S - 1), OP.max, OP.min
                )
                # round-to-nearest-int via the float magic-number trick
                ix = spool.tile([128, N], F32, tag=f"ix{ch}")
                RC = float(3 << 22)
                nc.vector.tensor_scalar(ix[:], tv[:], RC, RC, OP.add, OP.subtract)
                # transpose to [n(part), bt] for per-partition scalar operands
                for cc in range(NCHUNK):
                    pt = dppool.tile([GRID, 4 * GRID], F32, tag="dps")
                    nc.tensor.transpose(
                        pt[:, 0:128], ix[:, cc * 128 : (cc + 1) * 128], idt_sb[:]
                    )
                    st = cpool.tile([128, 128], F32, tag=f"T{ch}{cc}")
                    nc.scalar.activation(st[:], pt[:, 0:128], AT.Copy)
                    idxT[ch].append(st)

            # ---- main loop: 4 bt per PSUM-bank group, 3-stage software
            # pipeline so PE never blocks on ACT's PSUM->SBUF copies ----
            NG = BT_PER_CORE // 4
            w_sbs = [None] * NG
            u_sbs = [None] * NG
            d_pss = [None] * NG
            obufs = [None] * NG
            d_pss = [None] * NG
            u_pss = [None] * NG
            for i in range(NG + 3):
                # Stage spacing: Wcopy(g)@g, m1(g)@g+1, Ucopy(g)/m2(g)@g+2,
                # Dcopy(g)@g+3. ACT ops first (their PE deps finished in
                # earlier iterations); PE order Wmm -> m1 -> m2 so the
                # same-iteration ACT consumers (Wcopy, next iter's copies)
                # are fed early. No cross-engine dependency has less than
                # ~a full period of slack -> sem latency never accumulates.
                if 3 <= i:
                    g = i - 3
                    half = g % 2
                    if half == 0:
                        obuf = opool.tile([128, 8 * GRID], F32, tag="obuf")
                        obufs[g // 2] = obuf
                    nc.scalar.activation(
                        obufs[g // 2][:, half * 4 * GRID : (half + 1) * 4 * GRID],
                        d_pss[g][:], AT.Copy,
                    )
                    if half == 1:
                        nc.sync.dma_start(out_ext[g // 2], obufs[g // 2][:])
                if 2 <= i <= NG + 1:
                    g = i - 2
                    u_sb = usbpool.tile([S, 4 * GRID], BF16, tag="usb")
                    nc.scalar.activation(u_sb[:], u_pss[g][:], AT.Copy)
                    u_sbs[g] = u_sb
                if i == 0:
                    # GPSIMD one-hots run one group ahead so the W matmuls
                    # never stall PE waiting on the (slower) Pool engine.
                    ohps = [None] * NG
                    for gp in (0, 1):
                        tp = ohppool.tile([128, 8 * S], BF16, tag="ohp")
                        for k in range(4):
                            bt = 4 * gp + k
                            nc.gpsimd.tensor_scalar(
                                tp[:, 2 * k * S : (2 * k + 1) * S], iota_sb[:],
                                idxT["x"][3][:, bt : bt + 1], None, OP.is_equal,
                            )
                            nc.gpsimd.tensor_scalar(
                                tp[:, (2 * k + 1) * S : (2 * k + 2) * S], iota_sb[:],
                                idxT["y"][3][:, bt : bt + 1], None, OP.is_equal,
                            )
                        ohps[gp] = tp
                if i < NG:
                    g = i
                    w_ps = wppool.tile([S, 4 * S], F32, tag="wps")
                    # 6 DVE one-hots (x/y, chunks 0-2) share one tile so
                    # PSUM matmuls wait on a single engine clock and the
                    # buffer-recycle waits are per-tile, not per-slice.
                    ohd = [None] * 4
                    for k in range(4):
                        bt = 4 * g + k
                        td = ohpool.tile([128, 6 * S], BF16, tag="ohd")
                        for cc in range(3):
                            nc.vector.tensor_scalar(
                                td[:, cc * S : (cc + 1) * S], iota_sb[:],
                                idxT["x"][cc][:, bt : bt + 1], None, OP.is_equal,
                            )
                            nc.vector.tensor_scalar(
                                td[:, (3 + cc) * S : (4 + cc) * S], iota_sb[:],
                                idxT["y"][cc][:, bt : bt + 1], None, OP.is_equal,
                            )
                        ohd[k] = td
                    if g + 2 < NG:
                        gp = g + 2
                        tp = ohppool.tile([128, 8 * S], BF16, tag="ohp")
                        for k in range(4):
                            bt = 4 * gp + k
                            nc.gpsimd.tensor_scalar(
                                tp[:, 2 * k * S : (2 * k + 1) * S], iota_sb[:],
                                idxT["x"][3][:, bt : bt + 1], None, OP.is_equal,
                            )
                            nc.gpsimd.tensor_scalar(
                                tp[:, (2 * k + 1) * S : (2 * k + 2) * S], iota_sb[:],
                                idxT["y"][3][:, bt : bt + 1], None, OP.is_equal,
                            )
                        ohps[gp] = tp
                    for k in range(4):
                        for cc in range(3):
                            nc.tensor.matmul(
                                w_ps[:, k * S : (k + 1) * S],
                                ohd[k][:, cc * S : (cc + 1) * S],
                                ohd[k][:, (3 + cc) * S : (4 + cc) * S],
                                start=(cc == 0), stop=False,
                            )
                        nc.tensor.matmul(
                            w_ps[:, k * S : (k + 1) * S],
                            ohps[g][:, 2 * k * S : (2 * k + 1) * S],
                            ohps[g][:, (2 * k + 1) * S : (2 * k + 2) * S],
                            start=False, stop=True,
                        )
                if 1 <= i <= NG:
                    g = i - 1
                    u_ps = uppool.tile([S, 4 * GRID], F32, tag="ups")
                    for k in range(4):
                        nc.tensor.matmul(
                            u_ps[:, k * GRID : (k + 1) * GRID],
                            w_sbs[g][:, k * S : (k + 1) * S], k1_sb[:],
                            start=True, stop=True,
                        )
                    u_pss[g] = u_ps
                if 2 <= i <= NG + 1:
                    g = i - 2
                    d_ps = dppool.tile([GRID, 4 * GRID], F32, tag="dps")
                    for k in range(4):
                        nc.tensor.matmul(
                            d_ps[:, k * GRID : (k + 1) * GRID],
                            u_sbs[g][:, k * GRID : (k + 1) * GRID], k2_sb[:],
                            start=True, stop=True,
                        )
                    d_pss[g] = d_ps
                if i < NG:
                    g = i
                    w_sb = wsbpool.tile([S, 4 * S], BF16, tag="wsb")
                    nc.scalar.activation(w_sb[:], w_ps[:], AT.Copy)
                    w_sbs[g] = w_sb

    if not nc.is_finalized():
        nc.finalize()
    return nc


def _consts(bw: float):
    h = float(bw)
    norm = 1.0 / (2.0 * math.pi * h * h)
    s = np.linspace(-HALF, HALF, S, dtype=np.float64)
    xg = np.linspace(-5.0, 5.0, GRID, dtype=np.float64)
    K1 = np.exp(-0.5 * (s[:, None] - xg[None, :]) ** 2 / (h * h))
    k1 = K1.astype(bfloat16)
    k2 = (K1 * norm).astype(bfloat16)
    iota = np.broadcast_to(np.arange(S, dtype=np.float64), (128, S))
    iota = iota.astype(bfloat16).copy()
    idt = np.eye(128, dtype=np.float32)
    return iota, k1, k2, idt


def kernel(A: np.ndarray, bandwidth: np.ndarray) -> np.ndarray:
    A = np.asarray(A, dtype=np.float32)
    bw = float(np.asarray(bandwidth))
    key = round(bw, 9)
    if key not in _CACHE:
        _CACHE[key] = _build(bw)
    nc = _CACHE[key]

    iota, k1, k2, idt = _consts(bw)
    a_flat = A.reshape(B * T, N, 2)
    in_maps = []
    for i in range(NCORES):
        in_maps.append(
            {
                "a": np.ascontiguousarray(
                    a_flat[i * BT_PER_CORE : (i + 1) * BT_PER_CORE]
                ),
                "iota": iota,
                "k1": k1,
                "k2": k2,
                "idt": idt,
            }
        )
    res = run_bass_kernel_spmd(nc, in_maps, core_ids=list(range(NCORES)))
    outs = []
    for i in range(NCORES):
        o = res.results[i]["out"]  # [16, 128, 8*128]
        o = o.reshape(BT_PER_CORE // 8, GRID, 8, GRID)
        o = o.transpose(0, 2, 1, 3).reshape(BT_PER_CORE, GRID, GRID)
        outs.append(o)
    return np.concatenate(outs, axis=0).reshape(B, T, GRID, GRID)


if __name__ == "__main__":
    A = np.random.randn(B, T, N, 2).astype(np.float32)
    out = kernel(A, np.float32(0.5))
    print(out.shape, out.dtype, float(out.max()))


# revision 31
# speedup vs baseline: 1.0540x; 1.0540x over previous
"""KDE2D Trainium2 Bass kernel — nearest-binned separable formulation.

Reference (per (b,t), B=16, T=64, N=512, grid 128x128, h=bandwidth):
  standardize points (mean/std ddof=1 over N), then
  density[gx,gy] = norm * sum_n exp(-c(xg[gx]-x_n)^2) * exp(-c(yg[gy]-y_n)^2)

Kernel strategy (data-parallel over 1024 (b,t) pairs, 128 per core):
  Nearest-neighbour binning on an auxiliary S=96 grid s[-5.25, 5.25]:
    density ~= K1^T W K2,  W[i,j] = #{n : ix_n==i, iy_n==j},
    K1[i,g] = exp(-c(s_i-g_g)^2), K2 = K1*norm  (constants).
  Measured rel-Frobenius error of the approximation vs the exact
  reference is 6.7e-3 (tolerance 2e-2).

  Per (b,t): one-hot tiles U[cc][n(128 part), S] = (iota==idx_n) built by
  DVE tensor_scalar(is_equal) in 4x bf16 mode (94ns) / GPSIMD for one
  chunk pair; W accumulated over 4 n-chunks by PE; then two constant
  matmuls (st=W, mv=K1) -> U, (st=U, mv=K2) -> density. Four (b,t) share
  each PSUM bank so the PSUM->SBUF copies (ACT) are [*, 4*tile] wide.
  Output DMA batched 8 bt per descriptor-set.
"""

import math

import numpy as np
from ml_dtypes import bfloat16

import concourse.bass as bass
import concourse.bacc as bacc
import concourse.mybir as mybir
from concourse import tile
from concourse.bass_utils import run_bass_kernel_spmd

B, T, N, GRID = 16, 64, 512, 128
NCORES = 8
BT_PER_CORE = (B * T) // NCORES  # 128
NCHUNK = N // 128  # 4
S = 80           # auxiliary binning grid size
HALF = 5.25      # auxiliary grid spans [-HALF, HALF]
DS = 2.0 * HALF / (S - 1)

F32 = mybir.dt.float32
BF16 = mybir.dt.bfloat16

_CACHE = {}


def _build(bw: float):
    nc = bacc.Bacc("TRN2", target_bir_lowering=False)
    a_ext = nc.declare_dram_parameter("a", [BT_PER_CORE, N, 2], F32, isOutput=False)
    iota_ext = nc.declare_dram_parameter("iota", [128, S], BF16, isOutput=False)
    k1_ext = nc.declare_dram_parameter("k1", [S, GRID], BF16, isOutput=False)
    k2_ext = nc.declare_dram_parameter("k2", [S, GRID], BF16, isOutput=False)
    idt_ext = nc.declare_dram_parameter("idt", [128, 128], F32, isOutput=False)
    # out[g2, gx, (half,k,gy)] ; host reshapes to [128bt, 128, 128]
    out_ext = nc.declare_dram_parameter(
        "out", [BT_PER_CORE // 4, GRID, 4 * GRID], F32, isOutput=True
    )

    AT = mybir.ActivationFunctionType
    OP = mybir.AluOpType

    with tile.TileContext(nc) as tc:
        with (
            tc.tile_pool(name="const", bufs=1) as cpool,
            tc.tile_pool(name="stats", bufs=1) as spool,
            tc.tile_pool(name="work", bufs=4) as wpool,
            tc.tile_pool(name="oh", bufs=24) as ohpool,
            tc.tile_pool(name="ohp", bufs=32) as ohppool,
            tc.tile_pool(name="psumW", bufs=2, space="PSUM") as wppool,
            tc.tile_pool(name="psumU", bufs=3, space="PSUM") as uppool,
            tc.tile_pool(name="psumD", bufs=3, space="PSUM") as dppool,
            tc.tile_pool(name="wsb", bufs=4) as wsbpool,
            tc.tile_pool(name="usb", bufs=5) as usbpool,
            tc.tile_pool(name="outp", bufs=4) as opool,
        ):
            # split the input load across two DGE queues (SP + ACT) so the
            # halves transfer in parallel and stats can start on half 1
            a_all = spool.tile([128, N, 2], F32, tag="a")
            NH = N // 2
            nc.sync.dma_start(a_all[:, 0:NH, :], a_ext[:, 0:NH])
            nc.scalar.dma_start(a_all[:, NH:N, :], a_ext[:, NH:N])
            iota_sb = cpool.tile([128, S], BF16, tag="iota")
            k1_sb = cpool.tile([S, GRID], BF16, tag="k1")
            k2_sb = cpool.tile([S, GRID], BF16, tag="k2")
            idt_sb = cpool.tile([128, 128], F32, tag="idt")
            nc.sync.dma_start(iota_sb[:], iota_ext[:])
            nc.sync.dma_start(k1_sb[:], k1_ext[:])
            nc.sync.dma_start(k2_sb[:], k2_ext[:])
            nc.sync.dma_start(idt_sb[:], idt_ext[:])

            # ---- per-bt stats -> bin indices (layout [bt(128 part), n]) ----
            # t = ((x-mean)*invsd + HALF)/DS + 0.5 ; idx = floor(clip(t))
            #   = x*A + Bc with A = invsd/DS, Bc = (HALF - mean*invsd)/DS + .5
            dummy_pt = dppool.tile([GRID, 4 * GRID], F32, tag="dps")
            nc.tensor.transpose(dummy_pt[:, 0:128], idt_sb[:], idt_sb[:])
            probe = spool.tile([128, 1], F32, tag="probe")
            nc.scalar.activation(probe[:], iota_sb[:, 0:1], AT.Copy)
            idxT = {"x": [], "y": []}
            for ch, ci in (("x", 0), ("y", 1)):
                src = a_all[:, :, ci]
                bn = spool.tile([128, 2, 6], F32, tag=f"bn{ch}")
                nc.vector.bn_stats(bn[:, 0, :], a_all[:, 0:NH, ci])
                nc.vector.bn_stats(bn[:, 1, :], a_all[:, NH:N, ci])
                mv = spool.tile([128, 2], F32, tag=f"mv{ch}")
                nc.vector.bn_aggr(mv[:], bn[:])
                # invsd = 1/sqrt(var_pop * N/(N-1))  (ddof=1)
                sd = spool.tile([128, 1], F32, tag=f"sd{ch}")
                nc.scalar.activation(
                    sd[:], mv[:, 1:2], AT.Sqrt, scale=float(N) / (N - 1)
                )
                invsd = spool.tile([128, 1], F32, tag=f"invsd{ch}")
                nc.vector.reciprocal(invsd[:], sd[:])
                av = spool.tile([128, 1], F32, tag=f"av{ch}")
                nc.vector.tensor_scalar_mul(av[:], invsd[:], 1.0 / DS)
                mb = spool.tile([128, 1], F32, tag=f"mb{ch}")
                nc.vector.tensor_tensor(mb[:], mv[:, 0:1], av[:], OP.mult)
                bv = spool.tile([128, 1], F32, tag=f"bv{ch}")
                nc.vector.tensor_scalar(
                    bv[:], mb[:], -1.0, HALF / DS, OP.mult, OP.add
                )
                tv = wpool.tile([128, N], F32, tag=f"tv{ch}")
                nc.vector.tensor_scalar(
                    tv[:], src, av[:, 0:1], bv[:, 0:1], OP.mult, OP.add
                )
                # round-to-nearest-int via the float magic-number trick
                ix = spool.tile([128, N], F32, tag=f"ix{ch}")
                RC = float(3 << 22)
                nc.vector.tensor_scalar(ix[:], tv[:], RC, RC, OP.add, OP.subtract)
                # transpose to [n(part), bt] for per-partition scalar operands
                for cc in range(NCHUNK):
                    pt = dppool.tile([GRID, 4 * GRID], F32, tag="dps")
                    nc.tensor.transpose(
                        pt[:, 0:128], ix[:, cc * 128 : (cc + 1) * 128], idt_sb[:]
                    )
                    st = cpool.tile([128, 128], F32, tag=f"T{ch}{cc}")
                    nc.scalar.activation(st[:], pt[:, 0:128], AT.Copy)
                    idxT[ch].append(st)

            # ---- main loop: 4 bt per PSUM-bank group, 3-stage software
            # pipeline so PE never blocks on ACT's PSUM->SBUF copies ----
            NG = BT_PER_CORE // 4
            w_sbs = [None] * NG
            u_sbs = [None] * NG
            d_pss = [None] * NG
            obufs = [None] * NG
            d_pss = [None] * NG
            u_pss = [None] * NG
            for i in range(NG + 3):
                # Stage spacing: Wcopy(g)@g, m1(g)@g+1, Ucopy(g)/m2(g)@g+2,
                # Dcopy(g)@g+3. ACT ops first (their PE deps finished in
                # earlier iterations); PE order Wmm -> m1 -> m2 so the
                # same-iteration ACT consumers (Wcopy, next iter's copies)
                # are fed early. No cross-engine dependency has less than
                # ~a full period of slack -> sem latency never accumulates.
                if 3 <= i:
                    g = i - 3
                    d_ps = dppool.tile([GRID, 4 * GRID], F32, tag="dps")
                    for k in range(4):
                        nc.tensor.matmul(
                            d_ps[:, k * GRID : (k + 1) * GRID],
                            u_sbs[g][:, k * GRID : (k + 1) * GRID], k2_sb[:],
                            start=True, stop=True,
                        )
                    obuf = opool.tile([128, 4 * GRID], F32, tag="obuf")
                    nc.scalar.activation(obuf[:], d_ps[:], AT.Copy)
                    nc.sync.dma_start(out_ext[g], obuf[:])
                if i == 0:
                    # GPSIMD one-hots run one group ahead so the W matmuls
                    # never stall PE waiting on the (slower) Pool engine.
                    ohps = [None] * NG
                    for gp in (0, 1):
                        tp = ohppool.tile([128, 8 * S], BF16, tag="ohp")
                        for k in range(4):
                            bt = 4 * gp + k
                            nc.gpsimd.tensor_scalar(
                                tp[:, 2 * k * S : (2 * k + 1) * S], iota_sb[:],
                                idxT["x"][3][:, bt : bt + 1], None, OP.is_equal,
                            )
                            nc.gpsimd.tensor_scalar(
                                tp[:, (2 * k + 1) * S : (2 * k + 2) * S], iota_sb[:],
                                idxT["y"][3][:, bt : bt + 1], None, OP.is_equal,
                            )
                        ohps[gp] = tp
                if i < NG:
                    g = i
                    w_ps = wppool.tile([S, 4 * S], F32, tag="wps")
                    # 6 DVE one-hots (x/y, chunks 0-2) share one tile so
                    # PSUM matmuls wait on a single engine clock and the
                    # buffer-recycle waits are per-tile, not per-slice.
                    ohd = [None] * 4
                    for k in range(4):
                        bt = 4 * g + k
                        td = ohpool.tile([128, 6 * S], BF16, tag="ohd")
                        for cc in range(3):
                            nc.vector.tensor_scalar(
                                td[:, cc * S : (cc + 1) * S], iota_sb[:],
                                idxT["x"][cc][:, bt : bt + 1], None, OP.is_equal,
                            )
                            nc.vector.tensor_scalar(
                                td[:, (3 + cc) * S : (4 + cc) * S], iota_sb[:],
                                idxT["y"][cc][:, bt : bt + 1], None, OP.is_equal,
                            )
                        ohd[k] = td
                    if g + 2 < NG:
                        gp = g + 2
                        tp = ohppool.tile([128, 8 * S], BF16, tag="ohp")
                        for k in range(4):
                            bt = 4 * gp + k
                            nc.gpsimd.tensor_scalar(
                                tp[:, 2 * k * S : (2 * k + 1) * S], iota_sb[:],
                                idxT["x"][3][:, bt : bt + 1], None, OP.is_equal,
                            )
                            nc.gpsimd.tensor_scalar(
                                tp[:, (2 * k + 1) * S : (2 * k + 2) * S], iota_sb[:],
                                idxT["y"][3][:, bt : bt + 1], None, OP.is_equal,
                            )
                        ohps[gp] = tp
                    for k in range(4):
                        for cc in range(3):
                            nc.tensor.matmul(
                                w_ps[:, k * S : (k + 1) * S],
                                ohd[k][:, cc * S : (cc + 1) * S],
                                ohd[k][:, (3 + cc) * S : (4 + cc) * S],
                                start=(cc == 0), stop=False,
                            )
                        nc.tensor.matmul(
                            w_ps[:, k * S : (k + 1) * S],
                            ohps[g][:, 2 * k * S : (2 * k + 1) * S],
                            ohps[g][:, (2 * k + 1) * S : (2 * k + 2) * S],
                            start=False, stop=True,
                        )
                if 2 <= i <= NG + 1:
                    g = i - 2
                    u_ps = uppool.tile([S, 4 * GRID], F32, tag="ups")
                    for k in range(4):
                        nc.tensor.matmul(
                            u_ps[:, k * GRID : (k + 1) * GRID],
                            w_sbs[g][:, k * S : (k + 1) * S], k1_sb[:],
                            start=True, stop=True,
                        )
                    u_pss[g] = u_ps
                    u_sb = usbpool.tile([S, 4 * GRID], BF16, tag="usb")
                    nc.scalar.activation(u_sb[:], u_ps[:], AT.Copy)
                    u_sbs[g] = u_sb
                if i < NG:
                    g = i
                    w_sb = wsbpool.tile([S, 4 * S], BF16, tag="wsb")
                    nc.scalar.activation(w_sb[:], w_ps[:], AT.Copy)
                    w_sbs[g] = w_sb

    if not nc.is_finalized():
        nc.finalize()
    return nc


def _consts(bw: float):
    h = float(bw)
    norm = 1.0 / (2.0 * math.pi * h * h)
    s = np.linspace(-HALF, HALF, S, dtype=np.float64)
    xg = np.linspace(-5.0, 5.0, GRID, dtype=np.float64)
    K1 = np.exp(-0.5 * (s[:, None] - xg[None, :]) ** 2 / (h * h))
    k1 = K1.astype(bfloat16)
    k2 = (K1 * norm).astype(bfloat16)
    iota = np.broadcast_to(np.arange(S, dtype=np.float64), (128, S))
    iota = iota.astype(bfloat16).copy()
    idt = np.eye(128, dtype=np.float32)
    return iota, k1, k2, idt


def kernel(A: np.ndarray, bandwidth: np.ndarray) -> np.ndarray:
    A = np.asarray(A, dtype=np.float32)
    bw = float(np.asarray(bandwidth))
    key = round(bw, 9)
    if key not in _CACHE:
        _CACHE[key] = _build(bw)
    nc = _CACHE[key]

    iota, k1, k2, idt = _consts(bw)
    a_flat = A.reshape(B * T, N, 2)
    in_maps = []
    for i in range(NCORES):
        in_maps.append(
            {
                "a": np.ascontiguousarray(
                    a_flat[i * BT_PER_CORE : (i + 1) * BT_PER_CORE]
                ),
                "iota": iota,
                "k1": k1,
                "k2": k2,
                "idt": idt,
            }
        )
    res = run_bass_kernel_spmd(nc, in_maps, core_ids=list(range(NCORES)))
    outs = []
    for i in range(NCORES):
        o = res.results[i]["out"]  # [32, 128, 4*128]
        o = o.reshape(BT_PER_CORE // 4, GRID, 4, GRID)
        o = o.transpose(0, 2, 1, 3).reshape(BT_PER_CORE, GRID, GRID)
        outs.append(o)
    return np.concatenate(outs, axis=0).reshape(B, T, GRID, GRID)


if __name__ == "__main__":
    A = np.random.randn(B, T, N, 2).astype(np.float32)
    out = kernel(A, np.float32(0.5))
    print(out.shape, out.dtype, float(out.max()))


# revision 41
# speedup vs baseline: 1.1174x; 1.0602x over previous
"""KDE2D Trainium2 Bass kernel — nearest-binned separable formulation.

Reference (per (b,t), B=16, T=64, N=512, grid 128x128, h=bandwidth):
  standardize points (mean/std ddof=1 over N), then
  density[gx,gy] = norm * sum_n exp(-c(xg[gx]-x_n)^2) * exp(-c(yg[gy]-y_n)^2)

Kernel strategy (data-parallel over 1024 (b,t) pairs, 128 per core):
  Nearest-neighbour binning on an auxiliary S=96 grid s[-5.25, 5.25]:
    density ~= K1^T W K2,  W[i,j] = #{n : ix_n==i, iy_n==j},
    K1[i,g] = exp(-c(s_i-g_g)^2), K2 = K1*norm  (constants).
  Measured rel-Frobenius error of the approximation vs the exact
  reference is 6.7e-3 (tolerance 2e-2).

  Per (b,t): one-hot tiles U[cc][n(128 part), S] = (iota==idx_n) built by
  DVE tensor_scalar(is_equal) in 4x bf16 mode (94ns) / GPSIMD for one
  chunk pair; W accumulated over 4 n-chunks by PE; then two constant
  matmuls (st=W, mv=K1) -> U, (st=U, mv=K2) -> density. Four (b,t) share
  each PSUM bank so the PSUM->SBUF copies (ACT) are [*, 4*tile] wide.
  Output DMA batched 8 bt per descriptor-set.
"""

import math

import numpy as np
from ml_dtypes import bfloat16

import concourse.bass as bass
import concourse.bacc as bacc
import concourse.mybir as mybir
from concourse import tile
from concourse.bass_utils import run_bass_kernel_spmd

B, T, N, GRID = 16, 64, 512, 128
NCORES = 8
BT_PER_CORE = (B * T) // NCORES  # 128
NCHUNK = N // 128  # 4
S = 72           # auxiliary binning grid size
HALF = 5.25      # auxiliary grid spans [-HALF, HALF]
DS = 2.0 * HALF / (S - 1)

F32 = mybir.dt.float32
BF16 = mybir.dt.bfloat16

_CACHE = {}


def _build(bw: float):
    nc = bacc.Bacc("TRN2", target_bir_lowering=False)
    a_ext = nc.declare_dram_parameter("a", [BT_PER_CORE, N, 2], F32, isOutput=False)
    iota_ext = nc.declare_dram_parameter("iota", [128, S], BF16, isOutput=False)
    k1_ext = nc.declare_dram_parameter("k1", [S, GRID], BF16, isOutput=False)
    k2_ext = nc.declare_dram_parameter("k2", [S, GRID], BF16, isOutput=False)
    idt_ext = nc.declare_dram_parameter("idt", [128, 128], F32, isOutput=False)
    # out[g2, gx, (half,k,gy)] ; host reshapes to [128bt, 128, 128]
    out_ext = nc.declare_dram_parameter(
        "out", [BT_PER_CORE // 4, GRID, 4 * GRID], F32, isOutput=True
    )

    AT = mybir.ActivationFunctionType
    OP = mybir.AluOpType

    with tile.TileContext(nc) as tc:
        with (
            tc.tile_pool(name="const", bufs=1) as cpool,
            tc.tile_pool(name="stats", bufs=1) as spool,
            tc.tile_pool(name="work", bufs=4) as wpool,
            tc.tile_pool(name="oh", bufs=6) as ohpool,
            tc.tile_pool(name="ohp", bufs=32) as ohppool,
            tc.tile_pool(name="psumW", bufs=3, space="PSUM") as wppool,
            tc.tile_pool(name="psumU", bufs=3, space="PSUM") as uppool,
            tc.tile_pool(name="psumD", bufs=2, space="PSUM") as dppool,
            tc.tile_pool(name="wsb", bufs=4) as wsbpool,
            tc.tile_pool(name="usb", bufs=5) as usbpool,
            tc.tile_pool(name="outp", bufs=4) as opool,
        ):
            # split the input load across two DGE queues (SP + ACT) so the
            # halves transfer in parallel and stats can start on half 1
            a_all = spool.tile([128, N, 2], F32, tag="a")
            NH = N // 2
            NT = 176
            nc.sync.dma_start(a_all[:, 0:NT, :], a_ext[:, 0:NT])
            nc.scalar.dma_start(a_all[:, NT : 2 * NT, :], a_ext[:, NT : 2 * NT])
            nc.gpsimd.dma_start(a_all[:, 2 * NT : N, :], a_ext[:, 2 * NT : N])
            iota_sb = cpool.tile([128, S], BF16, tag="iota")
            k1_sb = cpool.tile([S, GRID], BF16, tag="k1")
            k2_sb = cpool.tile([S, GRID], BF16, tag="k2")
            idt_sb = cpool.tile([128, 128], F32, tag="idt")
            nc.sync.dma_start(iota_sb[:], iota_ext[:])
            nc.sync.dma_start(k1_sb[:], k1_ext[:])
            nc.sync.dma_start(k2_sb[:], k2_ext[:])
            nc.sync.dma_start(idt_sb[:], idt_ext[:])

            # ---- per-bt stats -> bin indices (layout [bt(128 part), n]) ----
            # t = ((x-mean)*invsd + HALF)/DS + 0.5 ; idx = floor(clip(t))
            #   = x*A + Bc with A = invsd/DS, Bc = (HALF - mean*invsd)/DS + .5
            dummy_pt = dppool.tile([GRID, 4 * GRID], F32, tag="dps")
            nc.tensor.transpose(dummy_pt[:, 0:128], idt_sb[:], idt_sb[:])
            probe = spool.tile([128, 1], F32, tag="probe")
            nc.scalar.activation(probe[:], iota_sb[:, 0:1], AT.Copy)
            idxT = {"x": [], "y": []}
            for ch, ci in (("x", 0), ("y", 1)):
                src = a_all[:, :, ci]
                bn = spool.tile([128, 2, 6], F32, tag=f"bn{ch}")
                nc.vector.bn_stats(bn[:, 0, :], a_all[:, 0:NH, ci])
                nc.vector.bn_stats(bn[:, 1, :], a_all[:, NH:N, ci])
                mv = spool.tile([128, 2], F32, tag=f"mv{ch}")
                nc.vector.bn_aggr(mv[:], bn[:])
                # invsd = 1/sqrt(var_pop * N/(N-1))  (ddof=1)
                sd = spool.tile([128, 1], F32, tag=f"sd{ch}")
                nc.scalar.activation(
                    sd[:], mv[:, 1:2], AT.Sqrt, scale=float(N) / (N - 1)
                )
                invsd = spool.tile([128, 1], F32, tag=f"invsd{ch}")
                nc.vector.reciprocal(invsd[:], sd[:])
                av = spool.tile([128, 1], F32, tag=f"av{ch}")
                nc.vector.tensor_scalar_mul(av[:], invsd[:], 1.0 / DS)
                mb = spool.tile([128, 1], F32, tag=f"mb{ch}")
                nc.vector.tensor_tensor(mb[:], mv[:, 0:1], av[:], OP.mult)
                bv = spool.tile([128, 1], F32, tag=f"bv{ch}")
                nc.vector.tensor_scalar(
                    bv[:], mb[:], -1.0, HALF / DS, OP.mult, OP.add
                )
                tv = wpool.tile([128, N], F32, tag=f"tv{ch}")
                nc.vector.tensor_scalar(
                    tv[:], src, av[:, 0:1], bv[:, 0:1], OP.mult, OP.add
                )
                # round-to-nearest-int via the float magic-number trick
                ix = spool.tile([128, N], F32, tag=f"ix{ch}")
                RC = float(3 << 22)
                nc.vector.tensor_scalar(ix[:], tv[:], RC, RC, OP.add, OP.subtract)
                # transpose to [n(part), bt] for per-partition scalar operands
                for cc in range(NCHUNK):
                    pt = dppool.tile([GRID, 4 * GRID], F32, tag="dps")
                    nc.tensor.transpose(
                        pt[:, 0:128], ix[:, cc * 128 : (cc + 1) * 128], idt_sb[:]
                    )
                    st = cpool.tile([128, 128], F32, tag=f"T{ch}{cc}")
                    nc.scalar.activation(st[:], pt[:, 0:128], AT.Copy)
                    idxT[ch].append(st)

            # ---- main loop: 4 bt per PSUM-bank group, 3-stage software
            # pipeline so PE never blocks on ACT's PSUM->SBUF copies ----
            NG = BT_PER_CORE // 4
            w_sbs = [None] * NG
            w_pss = [None] * NG
            u_sbs = [None] * NG
            d_pss = [None] * NG
            obufs = [None] * NG
            d_pss = [None] * NG
            u_pss = [None] * NG
            # GPSIMD builds every group's chunk-3 one-hot pair up front:
            # 52us of independent work that stays permanently ahead of the
            # W matmuls, so PE never blocks on the slower Pool engine.
            ohps = [None] * NG
            for gp in range(NG):
                tp = ohppool.tile([128, 9 * S], BF16, tag="ohp")
                for k in range(4):
                    bt = 4 * gp + k
                    nc.gpsimd.tensor_scalar(
                        tp[:, 2 * k * S : (2 * k + 1) * S], iota_sb[:],
                        idxT["x"][3][:, bt : bt + 1], None, OP.is_equal,
                    )
                    nc.gpsimd.tensor_scalar(
                        tp[:, (2 * k + 1) * S : (2 * k + 2) * S], iota_sb[:],
                        idxT["y"][3][:, bt : bt + 1], None, OP.is_equal,
                    )
                # 9th slice: k=0 chunk-2 y-side, balancing DVE at 23 ops
                nc.gpsimd.tensor_scalar(
                    tp[:, 8 * S : 9 * S], iota_sb[:],
                    idxT["y"][2][:, 4 * gp : 4 * gp + 1], None, OP.is_equal,
                )
                ohps[gp] = tp
            for i in range(NG + 4):
                # Stage spacing: Wmm(g)@g, Wcopy(g)@g+1, m1/Ucopy(g)@g+2,
                # m2/Dcopy(g)@g+3. ACT's first op each iteration (Wcopy of
                # the previous group) depends only on PE work that finished
                # last iteration, so ACT never idles behind the current
                # group's DVE-paced W matmuls; PE's m1/m2 likewise read
                # copies that are >= 1 iteration old.
                if 1 <= i <= NG:
                    g = i - 1
                    w_sb = wsbpool.tile([S, 4 * S], BF16, tag="wsb")
                    nc.scalar.activation(w_sb[:], w_pss[g][:], AT.Copy)
                    w_sbs[g] = w_sb
                if 2 <= i <= NG + 1:
                    g = i - 2
                    u_ps = uppool.tile([S, 4 * GRID], F32, tag="ups")
                    for k in range(4):
                        nc.tensor.matmul(
                            u_ps[:, k * GRID : (k + 1) * GRID],
                            w_sbs[g][:, k * S : (k + 1) * S], k1_sb[:],
                            start=True, stop=True,
                        )
                    u_sb = usbpool.tile([S, 4 * GRID], BF16, tag="usb")
                    nc.scalar.activation(u_sb[:], u_ps[:], AT.Copy)
                    u_sbs[g] = u_sb
                if 3 <= i <= NG + 2:
                    g = i - 3
                    d_ps = dppool.tile([GRID, 4 * GRID], F32, tag="dps")
                    for k in range(4):
                        nc.tensor.matmul(
                            d_ps[:, k * GRID : (k + 1) * GRID],
                            u_sbs[g][:, k * GRID : (k + 1) * GRID], k2_sb[:],
                            start=True, stop=True,
                        )
                    obuf = opool.tile([128, 4 * GRID], F32, tag="obuf")
                    nc.scalar.activation(obuf[:], d_ps[:], AT.Copy)
                    nc.sync.dma_start(out_ext[g], obuf[:])
                if i < NG:
                    g = i
                    w_ps = wppool.tile([S, 4 * S], F32, tag="wps")
                    w_pss[g] = w_ps
                    # all 24 DVE one-hots of the group share one tile:
                    # buffer-recycle needs one spilled sem per group, not
                    # four, keeping the DVE sequencer under its 70ns/instr
                    # budget.
                    td = ohpool.tile([128, 24 * S], BF16, tag="ohd")
                    ohd = [td[:, 6 * k * S : (6 * k + 6) * S] for k in range(4)]
                    for k in range(4):
                        bt = 4 * g + k
                        for cc in range(3):
                            nc.vector.tensor_scalar(
                                ohd[k][:, cc * S : (cc + 1) * S], iota_sb[:],
                                idxT["x"][cc][:, bt : bt + 1], None, OP.is_equal,
                            )
                            if k == 0 and cc == 2:
                                continue  # built by GPSIMD (9th pool slice)
                            nc.vector.tensor_scalar(
                                ohd[k][:, (3 + cc) * S : (4 + cc) * S], iota_sb[:],
                                idxT["y"][cc][:, bt : bt + 1], None, OP.is_equal,
                            )
                    for k in range(4):
                        for cc in range(3):
                            mv = (
                                ohps[g][:, 8 * S : 9 * S]
                                if (k == 0 and cc == 2)
                                else ohd[k][:, (3 + cc) * S : (4 + cc) * S]
                            )
                            nc.tensor.matmul(
                                w_ps[:, k * S : (k + 1) * S],
                                ohd[k][:, cc * S : (cc + 1) * S],
                                mv,
                                start=(cc == 0), stop=False,
                            )
                        nc.tensor.matmul(
                            w_ps[:, k * S : (k + 1) * S],
                            ohps[g][:, 2 * k * S : (2 * k + 1) * S],
                            ohps[g][:, (2 * k + 1) * S : (2 * k + 2) * S],
                            start=False, stop=True,
                        )


    if not nc.is_finalized():
        nc.finalize()
    return nc


def _consts(bw: float):
    h = float(bw)
    norm = 1.0 / (2.0 * math.pi * h * h)
    s = np.linspace(-HALF, HALF, S, dtype=np.float64)
    xg = np.linspace(-5.0, 5.0, GRID, dtype=np.float64)
    K1 = np.exp(-0.5 * (s[:, None] - xg[None, :]) ** 2 / (h * h))
    k1 = K1.astype(bfloat16)
    k2 = (K1 * norm).astype(bfloat16)
    iota = np.broadcast_to(np.arange(S, dtype=np.float64), (128, S))
    iota = iota.astype(bfloat16).copy()
    idt = np.eye(128, dtype=np.float32)
    return iota, k1, k2, idt


def kernel(A: np.ndarray, bandwidth: np.ndarray) -> np.ndarray:
    A = np.asarray(A, dtype=np.float32)
    bw = float(np.asarray(bandwidth))
    key = round(bw, 9)
    if key not in _CACHE:
        _CACHE[key] = _build(bw)
    nc = _CACHE[key]

    iota, k1, k2, idt = _consts(bw)
    a_flat = A.reshape(B * T, N, 2)
    in_maps = []
    for i in range(NCORES):
        in_maps.append(
            {
                "a": np.ascontiguousarray(
                    a_flat[i * BT_PER_CORE : (i + 1) * BT_PER_CORE]
                ),
                "iota": iota,
                "k1": k1,
                "k2": k2,
                "idt": idt,
            }
        )
    res = run_bass_kernel_spmd(nc, in_maps, core_ids=list(range(NCORES)))
    outs = []
    for i in range(NCORES):
        o = res.results[i]["out"]  # [32, 128, 4*128]
        o = o.reshape(BT_PER_CORE // 4, GRID, 4, GRID)
        o = o.transpose(0, 2, 1, 3).reshape(BT_PER_CORE, GRID, GRID)
        outs.append(o)
    return np.concatenate(outs, axis=0).reshape(B, T, GRID, GRID)


if __name__ == "__main__":
    A = np.random.randn(B, T, N, 2).astype(np.float32)
    out = kernel(A, np.float32(0.5))
    print(out.shape, out.dtype, float(out.max()))


# revision 42
# speedup vs baseline: 1.1186x; 1.0010x over previous
"""KDE2D Trainium2 Bass kernel — nearest-binned separable formulation.

Reference (per (b,t), B=16, T=64, N=512, grid 128x128, h=bandwidth):
  standardize points (mean/std ddof=1 over N), then
  density[gx,gy] = norm * sum_n exp(-c(xg[gx]-x_n)^2) * exp(-c(yg[gy]-y_n)^2)

Kernel strategy (data-parallel over 1024 (b,t) pairs, 128 per core):
  Nearest-neighbour binning on an auxiliary S=96 grid s[-5.25, 5.25]:
    density ~= K1^T W K2,  W[i,j] = #{n : ix_n==i, iy_n==j},
    K1[i,g] = exp(-c(s_i-g_g)^2), K2 = K1*norm  (constants).
  Measured rel-Frobenius error of the approximation vs the exact
  reference is 6.7e-3 (tolerance 2e-2).

  Per (b,t): one-hot tiles U[cc][n(128 part), S] = (iota==idx_n) built by
  DVE tensor_scalar(is_equal) in 4x bf16 mode (94ns) / GPSIMD for one
  chunk pair; W accumulated over 4 n-chunks by PE; then two constant
  matmuls (st=W, mv=K1) -> U, (st=U, mv=K2) -> density. Four (b,t) share
  each PSUM bank so the PSUM->SBUF copies (ACT) are [*, 4*tile] wide.
  Output DMA batched 8 bt per descriptor-set.
"""

import math

import numpy as np
from ml_dtypes import bfloat16

import concourse.bass as bass
import concourse.bacc as bacc
import concourse.mybir as mybir
from concourse import tile
from concourse.bass_utils import run_bass_kernel_spmd

B, T, N, GRID = 16, 64, 512, 128
NCORES = 8
BT_PER_CORE = (B * T) // NCORES  # 128
NCHUNK = N // 128  # 4
S = 80           # auxiliary binning grid size
HALF = 5.25      # auxiliary grid spans [-HALF, HALF]
DS = 2.0 * HALF / (S - 1)

F32 = mybir.dt.float32
BF16 = mybir.dt.bfloat16

_CACHE = {}


def _build(bw: float):
    nc = bacc.Bacc("TRN2", target_bir_lowering=False)
    a_ext = nc.declare_dram_parameter("a", [BT_PER_CORE, N, 2], F32, isOutput=False)
    iota_ext = nc.declare_dram_parameter("iota", [128, S], BF16, isOutput=False)
    k1_ext = nc.declare_dram_parameter("k1", [S, GRID], BF16, isOutput=False)
    k2_ext = nc.declare_dram_parameter("k2", [S, GRID], BF16, isOutput=False)
    idt_ext = nc.declare_dram_parameter("idt", [128, 128], F32, isOutput=False)
    # out[g2, gx, (half,k,gy)] ; host reshapes to [128bt, 128, 128]
    out_ext = nc.declare_dram_parameter(
        "out", [BT_PER_CORE // 4, GRID, 4 * GRID], F32, isOutput=True
    )

    AT = mybir.ActivationFunctionType
    OP = mybir.AluOpType

    with tile.TileContext(nc) as tc:
        with (
            tc.tile_pool(name="const", bufs=1) as cpool,
            tc.tile_pool(name="stats", bufs=1) as spool,
            tc.tile_pool(name="work", bufs=4) as wpool,
            tc.tile_pool(name="oh", bufs=6) as ohpool,
            tc.tile_pool(name="ohp", bufs=32) as ohppool,
            tc.tile_pool(name="psumW", bufs=3, space="PSUM") as wppool,
            tc.tile_pool(name="psumU", bufs=3, space="PSUM") as uppool,
            tc.tile_pool(name="psumD", bufs=2, space="PSUM") as dppool,
            tc.tile_pool(name="wsb", bufs=4) as wsbpool,
            tc.tile_pool(name="usb", bufs=5) as usbpool,
            tc.tile_pool(name="outp", bufs=4) as opool,
        ):
            # split the input load across two DGE queues (SP + ACT) so the
            # halves transfer in parallel and stats can start on half 1
            a_all = spool.tile([128, N, 2], F32, tag="a")
            NH = N // 2
            NT = 176
            nc.sync.dma_start(a_all[:, 0:NT, :], a_ext[:, 0:NT])
            nc.scalar.dma_start(a_all[:, NT : 2 * NT, :], a_ext[:, NT : 2 * NT])
            nc.gpsimd.dma_start(a_all[:, 2 * NT : N, :], a_ext[:, 2 * NT : N])
            iota_sb = cpool.tile([128, S], BF16, tag="iota")
            k1_sb = cpool.tile([S, GRID], BF16, tag="k1")
            k2_sb = cpool.tile([S, GRID], BF16, tag="k2")
            idt_sb = cpool.tile([128, 128], F32, tag="idt")
            nc.sync.dma_start(iota_sb[:], iota_ext[:])
            nc.sync.dma_start(k1_sb[:], k1_ext[:])
            nc.sync.dma_start(k2_sb[:], k2_ext[:])
            nc.sync.dma_start(idt_sb[:], idt_ext[:])

            # ---- per-bt stats -> bin indices (layout [bt(128 part), n]) ----
            # t = ((x-mean)*invsd + HALF)/DS + 0.5 ; idx = floor(clip(t))
            #   = x*A + Bc with A = invsd/DS, Bc = (HALF - mean*invsd)/DS + .5
            dummy_pt = dppool.tile([GRID, 4 * GRID], F32, tag="dps")
            nc.tensor.transpose(dummy_pt[:, 0:128], idt_sb[:], idt_sb[:])
            probe = spool.tile([128, 1], F32, tag="probe")
            nc.scalar.activation(probe[:], iota_sb[:, 0:1], AT.Copy)
            idxT = {"x": [], "y": []}
            for ch, ci in (("x", 0), ("y", 1)):
                src = a_all[:, :, ci]
                bn = spool.tile([128, 2, 6], F32, tag=f"bn{ch}")
                nc.vector.bn_stats(bn[:, 0, :], a_all[:, 0:NH, ci])
                nc.vector.bn_stats(bn[:, 1, :], a_all[:, NH:N, ci])
                mv = spool.tile([128, 2], F32, tag=f"mv{ch}")
                nc.vector.bn_aggr(mv[:], bn[:])
                # invsd = 1/sqrt(var_pop * N/(N-1))  (ddof=1)
                sd = spool.tile([128, 1], F32, tag=f"sd{ch}")
                nc.scalar.activation(
                    sd[:], mv[:, 1:2], AT.Sqrt, scale=float(N) / (N - 1)
                )
                invsd = spool.tile([128, 1], F32, tag=f"invsd{ch}")
                nc.vector.reciprocal(invsd[:], sd[:])
                av = spool.tile([128, 1], F32, tag=f"av{ch}")
                nc.vector.tensor_scalar_mul(av[:], invsd[:], 1.0 / DS)
                mb = spool.tile([128, 1], F32, tag=f"mb{ch}")
                nc.vector.tensor_tensor(mb[:], mv[:, 0:1], av[:], OP.mult)
                bv = spool.tile([128, 1], F32, tag=f"bv{ch}")
                nc.vector.tensor_scalar(
                    bv[:], mb[:], -1.0, HALF / DS, OP.mult, OP.add
                )
                tv = wpool.tile([128, N], F32, tag=f"tv{ch}")
                nc.vector.tensor_scalar(
                    tv[:], src, av[:, 0:1], bv[:, 0:1], OP.mult, OP.add
                )
                # round-to-nearest-int via the float magic-number trick
                ix = spool.tile([128, N], F32, tag=f"ix{ch}")
                RC = float(3 << 22)
                nc.vector.tensor_scalar(ix[:], tv[:], RC, RC, OP.add, OP.subtract)
                # transpose to [n(part), bt] for per-partition scalar operands
                for cc in range(NCHUNK):
                    pt = dppool.tile([GRID, 4 * GRID], F32, tag="dps")
                    nc.tensor.transpose(
                        pt[:, 0:128], ix[:, cc * 128 : (cc + 1) * 128], idt_sb[:]
                    )
                    st = cpool.tile([128, 128], F32, tag=f"T{ch}{cc}")
                    nc.scalar.activation(st[:], pt[:, 0:128], AT.Copy)
                    idxT[ch].append(st)

            # ---- main loop: 4 bt per PSUM-bank group, 3-stage software
            # pipeline so PE never blocks on ACT's PSUM->SBUF copies ----
            NG = BT_PER_CORE // 4
            w_sbs = [None] * NG
            w_pss = [None] * NG
            u_sbs = [None] * NG
            d_pss = [None] * NG
            obufs = [None] * NG
            d_pss = [None] * NG
            u_pss = [None] * NG
            # GPSIMD builds every group's chunk-3 one-hot pair up front:
            # 52us of independent work that stays permanently ahead of the
            # W matmuls, so PE never blocks on the slower Pool engine.
            ohps = [None] * NG
            for gp in range(NG):
                tp = ohppool.tile([128, 9 * S], BF16, tag="ohp")
                for k in range(4):
                    bt = 4 * gp + k
                    nc.gpsimd.tensor_scalar(
                        tp[:, 2 * k * S : (2 * k + 1) * S], iota_sb[:],
                        idxT["x"][3][:, bt : bt + 1], None, OP.is_equal,
                    )
                    nc.gpsimd.tensor_scalar(
                        tp[:, (2 * k + 1) * S : (2 * k + 2) * S], iota_sb[:],
                        idxT["y"][3][:, bt : bt + 1], None, OP.is_equal,
                    )
                # 9th slice: k=0 chunk-2 y-side, balancing DVE at 23 ops
                nc.gpsimd.tensor_scalar(
                    tp[:, 8 * S : 9 * S], iota_sb[:],
                    idxT["y"][2][:, 4 * gp : 4 * gp + 1], None, OP.is_equal,
                )
                ohps[gp] = tp
            for i in range(NG + 4):
                # Stage spacing: Wmm(g)@g, Wcopy(g)@g+1, m1/Ucopy(g)@g+2,
                # m2/Dcopy(g)@g+3. ACT's first op each iteration (Wcopy of
                # the previous group) depends only on PE work that finished
                # last iteration, so ACT never idles behind the current
                # group's DVE-paced W matmuls; PE's m1/m2 likewise read
                # copies that are >= 1 iteration old.
                if 1 <= i <= NG:
                    g = i - 1
                    w_sb = wsbpool.tile([S, 4 * S], BF16, tag="wsb")
                    nc.scalar.activation(w_sb[:], w_pss[g][:], AT.Copy)
                    w_sbs[g] = w_sb
                if 2 <= i <= NG + 1:
                    g = i - 2
                    u_ps = uppool.tile([S, 4 * GRID], F32, tag="ups")
                    for k in range(4):
                        nc.tensor.matmul(
                            u_ps[:, k * GRID : (k + 1) * GRID],
                            w_sbs[g][:, k * S : (k + 1) * S], k1_sb[:],
                            start=True, stop=True,
                        )
                    u_sb = usbpool.tile([S, 4 * GRID], BF16, tag="usb")
                    nc.scalar.activation(u_sb[:], u_ps[:], AT.Copy)
                    u_sbs[g] = u_sb
                if 3 <= i <= NG + 2:
                    g = i - 3
                    d_ps = dppool.tile([GRID, 4 * GRID], F32, tag="dps")
                    for k in range(4):
                        nc.tensor.matmul(
                            d_ps[:, k * GRID : (k + 1) * GRID],
                            u_sbs[g][:, k * GRID : (k + 1) * GRID], k2_sb[:],
                            start=True, stop=True,
                        )
                    obuf = opool.tile([128, 4 * GRID], F32, tag="obuf")
                    nc.scalar.activation(obuf[:], d_ps[:], AT.Copy)
                    nc.sync.dma_start(out_ext[g], obuf[:])
                if i < NG:
                    g = i
                    w_ps = wppool.tile([S, 4 * S], F32, tag="wps")
                    w_pss[g] = w_ps
                    # all 24 DVE one-hots of the group share one tile:
                    # buffer-recycle needs one spilled sem per group, not
                    # four, keeping the DVE sequencer under its 70ns/instr
                    # budget.
                    td = ohpool.tile([128, 24 * S], BF16, tag="ohd")
                    ohd = [td[:, 6 * k * S : (6 * k + 6) * S] for k in range(4)]
                    for k in range(4):
                        bt = 4 * g + k
                        for cc in range(3):
                            nc.vector.tensor_scalar(
                                ohd[k][:, cc * S : (cc + 1) * S], iota_sb[:],
                                idxT["x"][cc][:, bt : bt + 1], None, OP.is_equal,
                            )
                            if k == 0 and cc == 2:
                                continue  # built by GPSIMD (9th pool slice)
                            nc.vector.tensor_scalar(
                                ohd[k][:, (3 + cc) * S : (4 + cc) * S], iota_sb[:],
                                idxT["y"][cc][:, bt : bt + 1], None, OP.is_equal,
                            )
                    for k in range(4):
                        for cc in range(3):
                            mv = (
                                ohps[g][:, 8 * S : 9 * S]
                                if (k == 0 and cc == 2)
                                else ohd[k][:, (3 + cc) * S : (4 + cc) * S]
                            )
                            nc.tensor.matmul(
                                w_ps[:, k * S : (k + 1) * S],
                                ohd[k][:, cc * S : (cc + 1) * S],
                                mv,
                                start=(cc == 0), stop=False,
                            )
                        nc.tensor.matmul(
                            w_ps[:, k * S : (k + 1) * S],
                            ohps[g][:, 2 * k * S : (2 * k + 1) * S],
                            ohps[g][:, (2 * k + 1) * S : (2 * k + 2) * S],
                            start=False, stop=True,
                        )


    if not nc.is_finalized():
        nc.finalize()
    return nc


def _consts(bw: float):
    h = float(bw)
    norm = 1.0 / (2.0 * math.pi * h * h)
    s = np.linspace(-HALF, HALF, S, dtype=np.float64)
    xg = np.linspace(-5.0, 5.0, GRID, dtype=np.float64)
    K1 = np.exp(-0.5 * (s[:, None] - xg[None, :]) ** 2 / (h * h))
    k1 = K1.astype(bfloat16)
    k2 = (K1 * norm).astype(bfloat16)
    iota = np.broadcast_to(np.arange(S, dtype=np.float64), (128, S))
    iota = iota.astype(bfloat16).copy()
    idt = np.eye(128, dtype=np.float32)
    return iota, k1, k2, idt


def kernel(A: np.ndarray, bandwidth: np.ndarray) -> np.ndarray:
    A = np.asarray(A, dtype=np.float32)
    bw = float(np.asarray(bandwidth))
    key = round(bw, 9)
    if key not in _CACHE:
        _CACHE[key] = _build(bw)
    nc = _CACHE[key]

    iota, k1, k2, idt = _consts(bw)
    a_flat = A.reshape(B * T, N, 2)
    in_maps = []
    for i in range(NCORES):
        in_maps.append(
            {
                "a": np.ascontiguousarray(
                    a_flat[i * BT_PER_CORE : (i + 1) * BT_PER_CORE]
                ),
                "iota": iota,
                "k1": k1,
                "k2": k2,
                "idt": idt,
            }
        )
    res = run_bass_kernel_spmd(nc, in_maps, core_ids=list(range(NCORES)))
    outs = []
    for i in range(NCORES):
        o = res.results[i]["out"]  # [32, 128, 4*128]
        o = o.reshape(BT_PER_CORE // 4, GRID, 4, GRID)
        o = o.transpose(0, 2, 1, 3).reshape(BT_PER_CORE, GRID, GRID)
        outs.append(o)
    return np.concatenate(outs, axis=0).reshape(B, T, GRID, GRID)


if __name__ == "__main__":
    A = np.random.randn(B, T, N, 2).astype(np.float32)
    out = kernel(A, np.float32(0.5))
    print(out.shape, out.dtype, float(out.max()))
